# revision 21
# baseline (speedup 1.0000x reference)
"""LinearCapsPro forward on 8 TRN2 NeuronCores.

Math: out[b,c] = sqrt(u^T sigma u), u = W_c x_b, sigma = (W_c W_c^T + eps I)^-1.
Host-side fold: G_c = W_c W_c^T + eps I = L_c L_c^T  =>  u^T G^-1 u = ||L_c^-1 u||^2.
With W'_c = L_c^-1 W_c the device kernel is just v = x @ W'^T, then
out[b,c] = sqrt(sum_d v[b, c*16+d]^2) - one big matmul + square + group-sum + sqrt.

Sharding: data-parallel over batch (512 rows/core), W' replicated; no collectives.

Inputs ship as fp8e4 (x scaled by SX, W' by SW; the scale comes back out via the
ACT square's input-scale: (v*1/(SX*SW))^2 == v_true^2). fp8 halves the dominant
per-exec cost on this runtime - input staging into the NEFF - and halves HBM
traffic. Rel err vs the f64 reference is ~1e-2 (tolerance 2e-2), dominated by
fp8 quantization of x and W'.

Host pre-arranges both operands into the exact SBUF layout
(x_sb[p, k, m] = x^T[k*128+p, m]; w flat stripe-major, k-major within stripe)
so every DMA-in is a contiguous per-partition copy - no strided gathers.

Schedule (per core, TimelineSim span ~51.1us; PE busy 43.1us ~= the 42.7us
fp8 stream minimum):
  - x rides the SP HW-DGE ring (k-edges 0,1,2,4,8,12,16); w rides the GPSIMD
    SW-DGE ring (stripe-0 k-edges 0,2,4,8,12,16; stripes 1-3 one DMA each) so
    the two descriptor-gen paths run in parallel instead of serializing on
    one HWDGE (~625ns per dma_start). Edges tuned by TimelineSim sweep.
  - Uneven cd-stripes (464,464,480,192): the small LAST stripe shortens the
    terminal epilogue chain; 464-480 keeps PSUM tiles within one 2KB bank.
  - Compute loops stripe -> k(16) -> m(4 x 128 batch rows): 4 PSUM banks live
    per stripe (double-buffered across stripes = 8 banks). k-outer on early
    stripes (matches DMA arrival); m-outer on the last so m0-m2 epilogues
    overlap the remaining matmuls.
  - Epilogue per (stripe, m): ACT square (pre-scaled) psum->sbuf, DVE
    group-sum(16), then per-stripe ACT sqrt + out-DMA on the SP ring as soon
    as that stripe's capsules are final (only the last stripe's chain is
    terminal; ACT-ring DMA gens would serialize with the sqrts).

Rejected with measurements: DoubleRow fp8 (one 256-col LDWEIGHTS per matmul -
legalization emits LDW per MM, no stationary reuse - serializes with the
stream; HW-measured slower), fused x+w tiles (PE operand reads contend with
DMA writes to the same tile), finer/coarser piece edges, even stripes.
"""

import sys

import numpy as np

try:
    import concourse  # noqa: F401
except ImportError:  # fresh grading dir: concourse lives in the RL repo
    sys.path.insert(0, "/opt/trn_rl_repo")

B, F, C, D = 4096, 2048, 100, 16
N_CORES = 8
BL = B // N_CORES  # 512 batch rows per core
CD = C * D  # 1600
EPS = 1e-4
KT = F // 128  # 16 contraction tiles
MT = BL // 128  # 4 batch tiles per core
STRIPES = (464, 464, 480, 192)  # cd-stripe widths (29+29+30+12 capsules)
ST = len(STRIPES)
SX = 16.0  # fp8 pre-scale for x  (|x| < 6  -> |x*SX| < 96, fp8e4 max 240)
SW = 512.0  # fp8 pre-scale for W' (|W'| < 0.05 -> |W'*SW| < 24)

_OFFS = [sum(STRIPES[:i]) for i in range(ST + 1)]  # cd col offsets
_WO = [o * KT for o in _OFFS]  # flat w offsets

_cached_nc = None


def build_bass(repeat=1):
    """repeat>1 builds a NEFF with the compute body repeated (same output) -
    used only for launch-overhead-immune slope timing, never for grading."""
    import concourse.bacc as bacc
    import concourse.mybir as mybir
    import concourse.tile as tile

    fp8 = mybir.dt.float8e4
    f32 = mybir.dt.float32
    nc = bacc.Bacc("TRN2", target_bir_lowering=False, debug=False, num_devices=N_CORES)
    xT = nc.dram_tensor("xT", [128, KT, BL], fp8, kind="ExternalInput")
    wT = nc.dram_tensor("wT", [128, CD * KT], fp8, kind="ExternalInput")
    out = nc.dram_tensor("out", [BL, C], f32, kind="ExternalOutput")

    with tile.TileContext(nc) as tc:
        with (
            tc.tile_pool(name="xp", bufs=1) as xp,
            tc.tile_pool(name="wp", bufs=1) as wp,
            tc.tile_pool(name="ps", bufs=2, space="PSUM") as psp,
            tc.tile_pool(name="ep", bufs=4) as ep,
            tc.tile_pool(name="rp", bufs=1) as rp,
        ):
            xsb = xp.tile([128, KT, BL], fp8)
            wsb = wp.tile([128, CD * KT], fp8)
            for a, b in zip((0, 1, 2, 4, 8, 12, 16)[:-1], (0, 1, 2, 4, 8, 12, 16)[1:]):
                nc.sync.dma_start(xsb[:, a:b], xT[:, a:b])
            NS0 = STRIPES[0]
            for a, b in zip((0, 2, 4, 8, 12, 16)[:-1], (0, 2, 4, 8, 12, 16)[1:]):
                nc.gpsimd.dma_start(wsb[:, a * NS0 : b * NS0], wT[:, a * NS0 : b * NS0])
            for s in range(1, ST):
                nc.gpsimd.dma_start(wsb[:, _WO[s] : _WO[s + 1]], wT[:, _WO[s] : _WO[s + 1]])
            sq_scale = 1.0 / (SX * SW)
            for r in range(repeat):
                res = [
                    rp.tile([128, C], f32, tag=f"res{m}", name=f"res_r{r}_m{m}")
                    for m in range(MT)
                ]
                for s in range(ST):
                    NSs = STRIPES[s]
                    pss = [
                        psp.tile([128, NSs], f32, tag=f"ps{m}", name=f"ps_s{s}_m{m}")
                        for m in range(MT)
                    ]
                    if s < ST - 1:
                        order = [(k, m) for k in range(KT) for m in range(MT)]
                    else:
                        order = [(k, m) for m in range(MT) for k in range(KT)]
                    for k, m in order:
                        nc.tensor.matmul(
                            pss[m][:],
                            xsb[:, k, m * 128 : (m + 1) * 128],  # lhsT [K, M]
                            wsb[:, _WO[s] + k * NSs : _WO[s] + (k + 1) * NSs],
                            start=(k == 0),
                            stop=(k == KT - 1),
                        )
                    c0, c1 = _OFFS[s] // D, _OFFS[s + 1] // D
                    for m in range(MT):
                        sq = ep.tile([128, NSs], f32, tag="sq", name=f"sq_r{r}_{s}_{m}")
                        nc.scalar.activation(
                            sq[:],
                            pss[m][:],
                            mybir.ActivationFunctionType.Square,
                            scale=sq_scale,
                        )
                        nc.vector.reduce_sum(
                            res[m][:, c0:c1],
                            sq[:].rearrange("p (c d) -> p c d", d=D),
                            axis=mybir.AxisListType.X,
                        )
                        nc.scalar.sqrt(res[m][:, c0:c1], res[m][:, c0:c1])
                        nc.sync.dma_start(
                            out[m * 128 : (m + 1) * 128, c0:c1], res[m][:, c0:c1]
                        )
    nc.compile()
    return nc


def prep_inputs(x: np.ndarray, weight: np.ndarray):
    """Host-side fold + fp8 quantize + SBUF-layout pre-arrange + shard."""
    import ml_dtypes

    fp8 = ml_dtypes.float8_e4m3  # IEEE e4m3 (max 240) == TRN FP8_EXP4
    W64 = weight.astype(np.float64)  # [C, D, F]
    G = np.einsum("cdf,cef->cde", W64, W64)
    G[:, np.arange(D), np.arange(D)] += EPS
    L = np.linalg.cholesky(G)
    Wp = np.linalg.solve(L, W64).reshape(CD, F)  # L^-1 W : [CD, F]
    W8 = np.clip(Wp * SW, -240.0, 240.0).astype(fp8)  # [CD, F]
    X8 = np.clip(x.astype(np.float64) * SX, -240.0, 240.0).astype(fp8)  # [B, F]
    # flat w, stripe-major; within a stripe k-major:
    # w[p, _WO[s] + k*NSs + n] = W'[_OFFS[s]+n, k*128+p]
    blocks = []
    for s in range(ST):
        blk = W8[_OFFS[s] : _OFFS[s + 1]]  # [NSs, F]
        blocks.append(
            blk.reshape(STRIPES[s], KT, 128).transpose(2, 1, 0).reshape(128, -1)
        )
    w_sb = np.ascontiguousarray(np.concatenate(blocks, axis=1))  # [128, CD*KT]
    in_maps = []
    for i in range(N_CORES):
        xi = X8[i * BL : (i + 1) * BL]  # [BL, F]
        # x_sb[p, k, m] = x[m, k*128+p]
        x_sb = np.ascontiguousarray(xi.reshape(BL, KT, 128).transpose(2, 1, 0))
        in_maps.append({"xT": x_sb, "wT": w_sb})
    return in_maps


def kernel(x: np.ndarray, weight: np.ndarray) -> np.ndarray:
    global _cached_nc
    x = np.asarray(x)
    weight = np.asarray(weight)
    assert x.shape == (B, F) and weight.shape == (C, D, F), (x.shape, weight.shape)
    in_maps = prep_inputs(x, weight)
    if _cached_nc is None:
        _cached_nc = build_bass()
    from concourse.bass_utils import run_bass_kernel_spmd

    res = run_bass_kernel_spmd(_cached_nc, in_maps, core_ids=list(range(N_CORES)))
    return np.concatenate(
        [res.results[i]["out"] for i in range(N_CORES)], axis=0
    ).astype(np.float32)
